# revision 8
# baseline (speedup 1.0000x reference)
"""Trainium2 Bass kernel for nn_BlockDiagonalLayer.

Computes out[b, n*64+j] = sin(omega[n] * (sum_i x[b,n,i] * W[n,j,i] + bias[n,j]))
for B=2048, N=1024 networks, D_IN=D_OUT=64, sharded over 8 NeuronCores along N.

Per core (128 networks = 64 pairs), per pair p and batch-chunk of 1024:
  - PE:       v = Wbd_p^T @ xT  (block-diag [W_even; W_odd] stationary, fp32,
              host-packed, all 64 preloaded in SBUF; one matmul per 512 cols)
  - DVE/ACT:  f = v*s1 + bsc    (s1 = omega/2pi, bsc = s1*bias; PSUM read;
              alternates engines per tile -- Pool cannot read PSUM on TRN2)
  - Pool:     t = f + MAGIC     (rounds k = round(f) into mantissa)
  - Pool/DVE: q = (t - MAGIC) - f   (= k - f, exact; scalar_tensor_tensor)
  - ACT:      y = Sin(-2pi * q) -> bf16   (|2pi q| <= pi + eps: in-table)
  sin(-2pi(k-f)) = sin(2pi f - 2pi k) = sin(omega*(Wx+b)).  Bias folded into
  the f pass (no bias matmul); output returned as bf16 (tol 2e-2 >> 2^-9).
Host does layout-only transforms (transpose / block-diag packing / scalars).
"""

import numpy as np
import ml_dtypes

import concourse.bass as bass
import concourse.tile as tile
from concourse import bacc, mybir
from concourse.alu_op_type import AluOpType
from concourse.bass_utils import run_bass_kernel_spmd

B, N, D = 2048, 1024, 64
NCORES = 8
NS = N // NCORES          # 128 nets per core
PAIRS = NS // 2           # 64
MMW = 512                 # matmul moving free dim (fp32 max / one PSUM bank)
EW = 1024                 # elementwise tile width (2 PSUM banks)
PB = 2                    # pairs per x/y DMA transfer

TWO_PI = float(2.0 * np.pi)
INV_2PI = float(1.0 / (2.0 * np.pi))
MAGIC = float(1.5 * 2 ** 23)

F32 = mybir.dt.float32
BF16 = mybir.dt.bfloat16


def build_bass(repeat: int = 1):
    """Build the per-core Bass program (same NEFF on all 8 cores).

    repeat > 1 re-runs the whole main loop (idempotent writes) for timing.
    """
    nc = bacc.Bacc("TRN2", target_bir_lowering=False, debug=False,
                   num_devices=NCORES)
    xT_d = nc.dram_tensor("xT", [PAIRS, 128, B], F32, kind="ExternalInput")
    wbd_d = nc.dram_tensor("wbd", [128, PAIRS * 128], F32, kind="ExternalInput")
    bsc_d = nc.dram_tensor("bsc", [128, PAIRS], F32, kind="ExternalInput")
    s1_d = nc.dram_tensor("s1", [128, PAIRS], F32, kind="ExternalInput")
    yT_d = nc.dram_tensor("yT", [PAIRS, 128, B], BF16, kind="ExternalOutput")

    with tile.TileContext(nc) as tc:
        with (
            tc.tile_pool(name="aux", bufs=1) as aux_pool,
            tc.tile_pool(name="wconst", bufs=1) as wc_pool,
            tc.tile_pool(name="xin", bufs=3) as x_pool,
            tc.tile_pool(name="oout", bufs=3) as o_pool,
            tc.tile_pool(name="fp", bufs=4) as f_pool,
            tc.tile_pool(name="tp", bufs=3) as t_pool,
            tc.tile_pool(name="qp", bufs=3) as q_pool,
            tc.tile_pool(name="ps", bufs=4, space="PSUM") as psum_pool,
        ):
            # --- constants (loaded once) ---
            wc_sb = wc_pool.tile([128, PAIRS * 128], F32)
            for _c in range(8):
                _w = PAIRS * 128 // 8
                nc.sync.dma_start(wc_sb[:, _c * _w:(_c + 1) * _w],
                                  wbd_d[:, _c * _w:(_c + 1) * _w])
            bsc_sb = aux_pool.tile([128, PAIRS], F32)
            nc.gpsimd.dma_start(bsc_sb[:], bsc_d[:])
            s1_sb = aux_pool.tile([128, PAIRS], F32)
            nc.gpsimd.dma_start(s1_sb[:], s1_d[:])

            # --- main loop (optionally wrapped in a HW loop for timing) ---
            import contextlib
            rep_ctx = tc.For_i(0, repeat, 1) if repeat > 1 else contextlib.nullcontext()
            with rep_ctx:
                for p0 in range(0, PAIRS, PB):
                    xt = x_pool.tile([128, PB * B], F32)
                    nc.sync.dma_start(
                        xt[:].rearrange("p (a b) -> p a b", a=PB),
                        xT_d[p0:p0 + PB].rearrange("a p b -> p a b"))
                    outt = o_pool.tile([128, PB * B], BF16)
                    for a in range(PB):
                        p = p0 + a
                        wst = wc_sb[:, p * 128:(p + 1) * 128]
                        bscp = bsc_sb[:, p:p + 1]
                        s1p = s1_sb[:, p:p + 1]
                        for e in range(B // EW):
                            v = psum_pool.tile([128, EW], F32)
                            for h in range(EW // MMW):
                                lo = h * MMW
                                bcol = a * B + e * EW + lo
                                nc.tensor.matmul(
                                    v[:, lo:lo + MMW], wst,
                                    xt[:, bcol:bcol + MMW],
                                    start=True, stop=True)
                            f = f_pool.tile([128, EW], F32)
                            if e == 0:
                                nc.vector.tensor_scalar(
                                    f[:], v[:], s1p, bscp,
                                    op0=AluOpType.mult, op1=AluOpType.add)
                            else:
                                nc.scalar.activation(
                                    f[:], v[:],
                                    mybir.ActivationFunctionType.Identity,
                                    bias=bscp, scale=s1p)
                            t = t_pool.tile([128, EW], F32)
                            nc.gpsimd.tensor_scalar_add(t[:], f[:], MAGIC)
                            q = q_pool.tile([128, EW], F32)
                            nc.vector.scalar_tensor_tensor(
                                q[:], t[:], MAGIC, f[:],
                                op0=AluOpType.subtract, op1=AluOpType.subtract)
                            nc.scalar.activation(
                                outt[:, a * B + e * EW:a * B + (e + 1) * EW],
                                q[:], mybir.ActivationFunctionType.Sin,
                                bias=0.0, scale=-TWO_PI)
                    nc.scalar.dma_start(
                        yT_d[p0:p0 + PB].rearrange("a p b -> p a b"),
                        outt[:].rearrange("p (a b) -> p a b", a=PB))
    nc.compile()
    return nc


def prep_inputs(x, weights, bias, omega):
    """Host-side layout prep -> list of 8 per-core input dicts."""
    x3 = x.reshape(B, NCORES, NS, D)
    # xT_all[c, n, i, b] = x[b, c*128+n, i]; blocked for cache friendliness
    xT_all = np.empty((NCORES, NS, D, B), np.float32)
    BBLK = 128
    for b0 in range(0, B, BBLK):
        xT_all[:, :, :, b0:b0 + BBLK] = x3[b0:b0 + BBLK].transpose(1, 2, 3, 0)

    in_maps = []
    for c in range(NCORES):
        sl = slice(c * NS, (c + 1) * NS)
        wc = weights[sl]                       # [128, 64, 64] (j, i)
        wT = wc.transpose(0, 2, 1)             # [net, i, j]
        # block-diag stationary per pair: [i2, pair*128 + j2]
        wbd = np.zeros((PAIRS, 128, 128), np.float32)
        wbd[:, :D, :D] = wT[0::2]
        wbd[:, D:, D:] = wT[1::2]
        wbd_host = np.ascontiguousarray(
            wbd.transpose(1, 0, 2).reshape(128, PAIRS * 128))

        bc = bias[sl].astype(np.float32)       # [128, 64]
        bv = np.empty((PAIRS, 128), np.float32)
        bv[:, :D] = bc[0::2]
        bv[:, D:] = bc[1::2]

        oc = omega[sl].astype(np.float32)      # [128]
        s1 = np.empty((PAIRS, 128), np.float32)
        s1[:, :D] = (oc[0::2] * np.float32(INV_2PI))[:, None]
        s1[:, D:] = (oc[1::2] * np.float32(INV_2PI))[:, None]
        s1_host = np.ascontiguousarray(s1.T)   # [128, PAIRS]
        bsc_host = np.ascontiguousarray((s1 * bv).astype(np.float32).T)

        xT_c = np.ascontiguousarray(xT_all[c].reshape(PAIRS, 128, B))
        in_maps.append({"xT": xT_c, "wbd": wbd_host, "bsc": bsc_host,
                        "s1": s1_host})
    return in_maps


def assemble_output(results):
    """[8 cores] of yT [PAIRS, 128, B] bf16 -> full [B, N*D] fp32."""
    out = np.empty((B, N * D), np.float32)
    for c in range(NCORES):
        yy = results[c]["yT"].reshape(NS * D, B)
        ov = out[:, c * NS * D:(c + 1) * NS * D]
        for b0 in range(0, B, 128):
            ov[b0:b0 + 128, :] = yy[:, b0:b0 + 128].T.astype(np.float32)
    return out


_NC_CACHE = {}


def kernel(x, weights, bias, omega):
    x = np.ascontiguousarray(x, np.float32)
    weights = np.ascontiguousarray(weights, np.float32)
    bias = np.ascontiguousarray(bias, np.float32)
    omega = np.ascontiguousarray(omega, np.float32)

    if "nc" not in _NC_CACHE:
        _NC_CACHE["nc"] = build_bass()
    nc = _NC_CACHE["nc"]
    in_maps = prep_inputs(x, weights, bias, omega)
    res = run_bass_kernel_spmd(nc, in_maps, core_ids=list(range(NCORES)))
    return assemble_output(res.results)


# revision 11
# speedup vs baseline: 5.3686x; 5.3686x over previous
"""Trainium2 Bass kernel for nn_BlockDiagonalLayer.

Computes out[b, n*64+j] = sin(omega[n] * (sum_i x[b,n,i] * W[n,j,i] + bias[n,j]))
for B=2048, N=1024 networks, D_IN=D_OUT=64, sharded over 8 NeuronCores along N.

Per core (128 networks = 64 pairs), per pair p and batch-chunk of 1024:
  - PE:       v = Wbd_p^T @ xT  (block-diag [W_even; W_odd] stationary, fp32,
              host-packed, all 64 preloaded in SBUF; one matmul per 512 cols)
  - DVE/ACT:  f = v*s1 + bsc    (s1 = omega/2pi, bsc = s1*bias; PSUM read;
              alternates engines per tile -- Pool cannot read PSUM on TRN2)
  - Pool:     t = f + MAGIC     (rounds k = round(f) into mantissa)
  - Pool/DVE: q = (t - MAGIC) - f   (= k - f, exact; scalar_tensor_tensor)
  - ACT:      y = Sin(-2pi * q) -> bf16   (|2pi q| <= pi + eps: in-table)
  sin(-2pi(k-f)) = sin(2pi f - 2pi k) = sin(omega*(Wx+b)).  Bias folded into
  the f pass (no bias matmul); output returned as bf16 (tol 2e-2 >> 2^-9).
Host does layout-only transforms (transpose / block-diag packing / scalars).
"""

import numpy as np
import ml_dtypes

import concourse.bass as bass
import concourse.tile as tile
from concourse import bacc, mybir
from concourse.alu_op_type import AluOpType
from concourse.bass_utils import run_bass_kernel_spmd

B, N, D = 2048, 1024, 64
NCORES = 8
NS = N // NCORES          # 128 nets per core
PAIRS = NS // 2           # 64
MMW = 512                 # matmul moving free dim (fp32 max / one PSUM bank)
EW = 1024                 # elementwise tile width (2 PSUM banks)
PB = 2                    # pairs per x/y DMA transfer

TWO_PI = float(2.0 * np.pi)
INV_2PI = float(1.0 / (2.0 * np.pi))
MAGIC = float(1.5 * 2 ** 23)

F32 = mybir.dt.float32
BF16 = mybir.dt.bfloat16


def build_bass(repeat: int = 1):
    """Build the per-core Bass program (same NEFF on all 8 cores).

    repeat > 1 re-runs the whole main loop (idempotent writes) for timing.
    """
    nc = bacc.Bacc("TRN2", target_bir_lowering=False, debug=False,
                   num_devices=NCORES)
    xT_d = nc.dram_tensor("xT", [PAIRS, 128, B], F32, kind="ExternalInput")
    wbd_d = nc.dram_tensor("wbd", [128, PAIRS * 128], F32, kind="ExternalInput")
    bsc_d = nc.dram_tensor("bsc", [128, PAIRS], F32, kind="ExternalInput")
    s1_d = nc.dram_tensor("s1", [128, PAIRS], F32, kind="ExternalInput")
    yT_d = nc.dram_tensor("yT", [PAIRS, 128, B], BF16, kind="ExternalOutput")

    with tile.TileContext(nc) as tc:
        with (
            tc.tile_pool(name="aux", bufs=1) as aux_pool,
            tc.tile_pool(name="wconst", bufs=1) as wc_pool,
            tc.tile_pool(name="xin", bufs=3) as x_pool,
            tc.tile_pool(name="oout", bufs=3) as o_pool,
            tc.tile_pool(name="fp", bufs=4) as f_pool,
            tc.tile_pool(name="tp", bufs=3) as t_pool,
            tc.tile_pool(name="qp", bufs=3) as q_pool,
            tc.tile_pool(name="ps", bufs=4, space="PSUM") as psum_pool,
        ):
            # --- constants (loaded once) ---
            wc_sb = wc_pool.tile([128, PAIRS * 128], F32)
            for _c in range(8):
                _w = PAIRS * 128 // 8
                nc.sync.dma_start(wc_sb[:, _c * _w:(_c + 1) * _w],
                                  wbd_d[:, _c * _w:(_c + 1) * _w])
            bsc_sb = aux_pool.tile([128, PAIRS], F32)
            nc.gpsimd.dma_start(bsc_sb[:], bsc_d[:])
            s1_sb = aux_pool.tile([128, PAIRS], F32)
            nc.gpsimd.dma_start(s1_sb[:], s1_d[:])
            magic_sb = aux_pool.tile([128, 1], F32)
            nc.gpsimd.memset(magic_sb[:], MAGIC)

            # --- main loop (optionally wrapped in a HW loop for timing) ---
            import contextlib
            rep_ctx = tc.For_i(0, repeat, 1) if repeat > 1 else contextlib.nullcontext()
            with rep_ctx:
                for p0 in range(0, PAIRS, PB):
                    xt = x_pool.tile([128, PB * B], F32)
                    nc.sync.dma_start(
                        xt[:].rearrange("p (a b) -> p a b", a=PB),
                        xT_d[p0:p0 + PB].rearrange("a p b -> p a b"))
                    outt = o_pool.tile([128, PB * B], BF16)
                    for a in range(PB):
                        p = p0 + a
                        wst = wc_sb[:, p * 128:(p + 1) * 128]
                        bscp = bsc_sb[:, p:p + 1]
                        s1p = s1_sb[:, p:p + 1]
                        for e in range(B // EW):
                            v = psum_pool.tile([128, EW], F32)
                            for h in range(EW // MMW):
                                lo = h * MMW
                                bcol = a * B + e * EW + lo
                                nc.tensor.matmul(
                                    v[:, lo:lo + MMW], wst,
                                    xt[:, bcol:bcol + MMW],
                                    start=True, stop=True)
                            f = f_pool.tile([128, EW], F32)
                            if e == 0:
                                nc.vector.tensor_scalar(
                                    f[:], v[:], s1p, bscp,
                                    op0=AluOpType.mult, op1=AluOpType.add)
                            else:
                                nc.scalar.activation(
                                    f[:], v[:],
                                    mybir.ActivationFunctionType.Identity,
                                    bias=bscp, scale=s1p)
                            t = t_pool.tile([128, EW], F32)
                            if e == 0:
                                nc.scalar.activation(
                                    t[:], f[:],
                                    mybir.ActivationFunctionType.Identity,
                                    bias=magic_sb[:], scale=1.0)
                            else:
                                nc.vector.tensor_scalar_add(t[:], f[:], MAGIC)
                            q = q_pool.tile([128, EW], F32)
                            nc.vector.scalar_tensor_tensor(
                                q[:], t[:], MAGIC, f[:],
                                op0=AluOpType.subtract, op1=AluOpType.subtract)
                            nc.scalar.activation(
                                outt[:, a * B + e * EW:a * B + (e + 1) * EW],
                                q[:], mybir.ActivationFunctionType.Sin,
                                bias=0.0, scale=-TWO_PI)
                    nc.scalar.dma_start(
                        yT_d[p0:p0 + PB].rearrange("a p b -> p a b"),
                        outt[:].rearrange("p (a b) -> p a b", a=PB))
    nc.compile()
    return nc


def prep_inputs(x, weights, bias, omega):
    """Host-side layout prep -> list of 8 per-core input dicts."""
    x3 = x.reshape(B, NCORES, NS, D)
    # xT_all[c, n, i, b] = x[b, c*128+n, i]; blocked for cache friendliness
    xT_all = np.empty((NCORES, NS, D, B), np.float32)
    BBLK = 128
    for b0 in range(0, B, BBLK):
        xT_all[:, :, :, b0:b0 + BBLK] = x3[b0:b0 + BBLK].transpose(1, 2, 3, 0)

    in_maps = []
    for c in range(NCORES):
        sl = slice(c * NS, (c + 1) * NS)
        wc = weights[sl]                       # [128, 64, 64] (j, i)
        wT = wc.transpose(0, 2, 1)             # [net, i, j]
        # block-diag stationary per pair: [i2, pair*128 + j2]
        wbd = np.zeros((PAIRS, 128, 128), np.float32)
        wbd[:, :D, :D] = wT[0::2]
        wbd[:, D:, D:] = wT[1::2]
        wbd_host = np.ascontiguousarray(
            wbd.transpose(1, 0, 2).reshape(128, PAIRS * 128))

        bc = bias[sl].astype(np.float32)       # [128, 64]
        bv = np.empty((PAIRS, 128), np.float32)
        bv[:, :D] = bc[0::2]
        bv[:, D:] = bc[1::2]

        oc = omega[sl].astype(np.float32)      # [128]
        s1 = np.empty((PAIRS, 128), np.float32)
        s1[:, :D] = (oc[0::2] * np.float32(INV_2PI))[:, None]
        s1[:, D:] = (oc[1::2] * np.float32(INV_2PI))[:, None]
        s1_host = np.ascontiguousarray(s1.T)   # [128, PAIRS]
        bsc_host = np.ascontiguousarray((s1 * bv).astype(np.float32).T)

        xT_c = np.ascontiguousarray(xT_all[c].reshape(PAIRS, 128, B))
        in_maps.append({"xT": xT_c, "wbd": wbd_host, "bsc": bsc_host,
                        "s1": s1_host})
    return in_maps


def assemble_output(results):
    """[8 cores] of yT [PAIRS, 128, B] bf16 -> full [B, N*D] fp32."""
    out = np.empty((B, N * D), np.float32)
    for c in range(NCORES):
        yy = results[c]["yT"].reshape(NS * D, B)
        ov = out[:, c * NS * D:(c + 1) * NS * D]
        for b0 in range(0, B, 128):
            ov[b0:b0 + 128, :] = yy[:, b0:b0 + 128].T.astype(np.float32)
    return out


_NC_CACHE = {}


def kernel(x, weights, bias, omega):
    x = np.ascontiguousarray(x, np.float32)
    weights = np.ascontiguousarray(weights, np.float32)
    bias = np.ascontiguousarray(bias, np.float32)
    omega = np.ascontiguousarray(omega, np.float32)

    if "nc" not in _NC_CACHE:
        _NC_CACHE["nc"] = build_bass()
    nc = _NC_CACHE["nc"]
    in_maps = prep_inputs(x, weights, bias, omega)
    res = run_bass_kernel_spmd(nc, in_maps, core_ids=list(range(NCORES)))
    return assemble_output(res.results)


# revision 15
# speedup vs baseline: 6.4993x; 1.2106x over previous
"""Trainium2 Bass kernel for nn_BlockDiagonalLayer.

Computes out[b, n*64+j] = sin(omega[n] * (sum_i x[b,n,i] * W[n,j,i] + bias[n,j]))
for B=2048, N=1024 networks, D_IN=D_OUT=64, sharded over 8 NeuronCores along N.

Per core (128 networks = 64 pairs), per pair p and batch-chunk of 1024:
  - PE:       f = Wbd_p'^T @ xT + bsc   where W' = W * (omega/2pi) is
              host-prescaled (block-diag [W'_even; W'_odd] fp32 stationary,
              all 64 preloaded in SBUF) and bsc = (omega/2pi)*bias enters via
              a tiny bf16 hi/lo bias-matmul against a ones vector.
  - DVE/ACT:  t = f + MAGIC             (rounds k = round(f) into mantissa;
              alternates DVE tensor_scalar / ACT Identity per tile)
  - DVE:      q = (t - MAGIC) - f       (= k - f, exact; scalar_tensor_tensor)
  - ACT:      y = Sin(-2pi * q) -> bf16 (|2pi q| <= pi + eps: in-table)
  sin(-2pi(k-f)) = sin(2pi f - 2pi k) = sin(omega*(Wx+b)).
Host does layout-only transforms (transpose / block-diag packing / scalars).
gpsimd (Pool/Q7) is used ONLY for setup DMAs/memsets: its elementwise
tensor ops cost ~13us per 1024-wide instruction on HW (software Q7 loop).
"""

import numpy as np
import ml_dtypes

import concourse.bass as bass
import concourse.tile as tile
from concourse import bacc, mybir
from concourse.alu_op_type import AluOpType
from concourse.bass_utils import run_bass_kernel_spmd

B, N, D = 2048, 1024, 64
NCORES = 8
NS = N // NCORES          # 128 nets per core
PAIRS = NS // 2           # 64
MMW = 512                 # matmul moving free dim (fp32 max / one PSUM bank)
EW = 1024                 # elementwise tile width (2 PSUM banks)
PB = 2                    # pairs per x/y DMA transfer

TWO_PI = float(2.0 * np.pi)
INV_2PI = float(1.0 / (2.0 * np.pi))
MAGIC = float(1.5 * 2 ** 23)

F32 = mybir.dt.float32
FP16 = mybir.dt.float16
BF16 = mybir.dt.bfloat16

MM_FP16 = True            # fp16 hi/lo 3-matmul (1 cyc/row) vs fp32 (4 cyc/row)


def build_bass(repeat: int = 1):
    """Build the per-core Bass program (same NEFF on all 8 cores).

    repeat > 1 re-runs the whole main loop (idempotent writes) for timing.
    """
    nc = bacc.Bacc("TRN2", target_bir_lowering=False, debug=False,
                   num_devices=NCORES)
    if MM_FP16:
        xh_d = nc.dram_tensor("xh", [PAIRS, 128, B], FP16, kind="ExternalInput")
        xl_d = nc.dram_tensor("xl", [PAIRS, 128, B], FP16, kind="ExternalInput")
        whl_d = nc.dram_tensor("whl", [128, PAIRS * 256], FP16,
                               kind="ExternalInput")
    else:
        xT_d = nc.dram_tensor("xT", [PAIRS, 128, B], F32, kind="ExternalInput")
        wbd_d = nc.dram_tensor("wbd", [128, PAIRS * 128], F32,
                               kind="ExternalInput")
    b2_d = nc.dram_tensor("b2", [4, PAIRS * 128], BF16, kind="ExternalInput")
    yT_d = nc.dram_tensor("yT", [PAIRS, 128, B], BF16, kind="ExternalOutput")

    with tile.TileContext(nc) as tc:
        with (
            tc.tile_pool(name="aux", bufs=1) as aux_pool,
            tc.tile_pool(name="wconst", bufs=1) as wc_pool,
            tc.tile_pool(name="xin", bufs=3) as x_pool,
            tc.tile_pool(name="oout", bufs=3) as o_pool,
            tc.tile_pool(name="tp", bufs=4) as t_pool,
            tc.tile_pool(name="qp", bufs=4) as q_pool,
            tc.tile_pool(name="ps", bufs=4, space="PSUM") as psum_pool,
        ):
            # --- constants (loaded once) ---
            if MM_FP16:
                wc_sb = wc_pool.tile([128, PAIRS * 256], FP16)
                for _c in range(8):
                    _w = PAIRS * 256 // 8
                    nc.sync.dma_start(wc_sb[:, _c * _w:(_c + 1) * _w],
                                      whl_d[:, _c * _w:(_c + 1) * _w])
            else:
                wc_sb = wc_pool.tile([128, PAIRS * 128], F32)
                for _c in range(8):
                    _w = PAIRS * 128 // 8
                    nc.sync.dma_start(wc_sb[:, _c * _w:(_c + 1) * _w],
                                      wbd_d[:, _c * _w:(_c + 1) * _w])
            b2_sb = aux_pool.tile([4, PAIRS * 128], BF16)
            nc.gpsimd.dma_start(b2_sb[:], b2_d[:])
            ones2 = aux_pool.tile([4, MMW], BF16)
            nc.gpsimd.memset(ones2[:], 1.0)
            magic_sb = aux_pool.tile([128, 1], F32)
            nc.gpsimd.memset(magic_sb[:], MAGIC)

            # --- main loop (optionally wrapped in a HW loop for timing) ---
            import contextlib
            rep_ctx = tc.For_i(0, repeat, 1) if repeat > 1 else contextlib.nullcontext()
            with rep_ctx:
                for p0 in range(0, PAIRS, PB):
                    if MM_FP16:
                        xt = x_pool.tile([128, PB * B], FP16, tag="xh")
                        nc.sync.dma_start(
                            xt[:].rearrange("p (a b) -> p a b", a=PB),
                            xh_d[p0:p0 + PB].rearrange("a p b -> p a b"))
                        xlt = x_pool.tile([128, PB * B], FP16, tag="xl")
                        nc.sync.dma_start(
                            xlt[:].rearrange("p (a b) -> p a b", a=PB),
                            xl_d[p0:p0 + PB].rearrange("a p b -> p a b"))
                    else:
                        xt = x_pool.tile([128, PB * B], F32)
                        nc.sync.dma_start(
                            xt[:].rearrange("p (a b) -> p a b", a=PB),
                            xT_d[p0:p0 + PB].rearrange("a p b -> p a b"))
                    outt = o_pool.tile([128, PB * B], BF16)
                    for a in range(PB):
                        p = p0 + a
                        b2t = b2_sb[:, p * 128:(p + 1) * 128]
                        if MM_FP16:
                            wh = wc_sb[:, p * 256:p * 256 + 128]
                            wl = wc_sb[:, p * 256 + 128:(p + 1) * 256]
                        else:
                            wst = wc_sb[:, p * 128:(p + 1) * 128]
                        for e in range(B // EW):
                            v = psum_pool.tile([128, EW], F32)
                            for h in range(EW // MMW):
                                lo = h * MMW
                                bcol = a * B + e * EW + lo
                                if MM_FP16:
                                    nc.tensor.matmul(
                                        v[:, lo:lo + MMW], wh,
                                        xt[:, bcol:bcol + MMW],
                                        start=True, stop=False)
                                    nc.tensor.matmul(
                                        v[:, lo:lo + MMW], wh,
                                        xlt[:, bcol:bcol + MMW],
                                        start=False, stop=False)
                                    nc.tensor.matmul(
                                        v[:, lo:lo + MMW], wl,
                                        xt[:, bcol:bcol + MMW],
                                        start=False, stop=False)
                                else:
                                    nc.tensor.matmul(
                                        v[:, lo:lo + MMW], wst,
                                        xt[:, bcol:bcol + MMW],
                                        start=True, stop=False)
                                nc.tensor.matmul(
                                    v[:, lo:lo + MMW], b2t, ones2[:],
                                    start=False, stop=True)
                            t = t_pool.tile([128, EW], F32)
                            if e == 0:
                                nc.vector.tensor_scalar_add(t[:], v[:], MAGIC)
                            else:
                                nc.scalar.activation(
                                    t[:], v[:],
                                    mybir.ActivationFunctionType.Identity,
                                    bias=magic_sb[:], scale=1.0)
                            q = q_pool.tile([128, EW], F32)
                            nc.vector.scalar_tensor_tensor(
                                q[:], t[:], MAGIC, v[:],
                                op0=AluOpType.subtract, op1=AluOpType.subtract)
                            nc.scalar.activation(
                                outt[:, a * B + e * EW:a * B + (e + 1) * EW],
                                q[:], mybir.ActivationFunctionType.Sin,
                                bias=0.0, scale=-TWO_PI)
                    nc.scalar.dma_start(
                        yT_d[p0:p0 + PB].rearrange("a p b -> p a b"),
                        outt[:].rearrange("p (a b) -> p a b", a=PB))
    nc.compile()
    return nc


def prep_inputs(x, weights, bias, omega):
    """Host-side layout prep -> list of 8 per-core input dicts."""
    bf16 = ml_dtypes.bfloat16
    x3 = x.reshape(B, NCORES, NS, D)
    # xT_all[c, n, i, b] = x[b, c*128+n, i]; blocked for cache friendliness
    xT_all = np.empty((NCORES, NS, D, B), np.float32)
    BBLK = 128
    for b0 in range(0, B, BBLK):
        xT_all[:, :, :, b0:b0 + BBLK] = x3[b0:b0 + BBLK].transpose(1, 2, 3, 0)

    s1_full = (omega.astype(np.float32) * np.float32(INV_2PI))
    if MM_FP16:
        xh_all = xT_all.astype(np.float16)
        xl_all = (xT_all - xh_all.astype(np.float32)).astype(np.float16)
    in_maps = []
    for c in range(NCORES):
        sl = slice(c * NS, (c + 1) * NS)
        s1c = s1_full[sl]                      # [128]
        # prescale: W'[n,j,i] = W[n,j,i] * s1[n]
        wc = (weights[sl] * s1c[:, None, None]).astype(np.float32)
        wT = wc.transpose(0, 2, 1)             # [net, i, j]
        if MM_FP16:
            whl = np.zeros((PAIRS, 2, 128, 128), np.float16)
            wT_h = wT.astype(np.float16)
            wT_l = (wT - wT_h.astype(np.float32)).astype(np.float16)
            whl[:, 0, :D, :D] = wT_h[0::2]
            whl[:, 0, D:, D:] = wT_h[1::2]
            whl[:, 1, :D, :D] = wT_l[0::2]
            whl[:, 1, D:, D:] = wT_l[1::2]
            whl_host = np.ascontiguousarray(
                whl.transpose(2, 0, 1, 3).reshape(128, PAIRS * 256))
        else:
            wbd = np.zeros((PAIRS, 128, 128), np.float32)
            wbd[:, :D, :D] = wT[0::2]
            wbd[:, D:, D:] = wT[1::2]
            wbd_host = np.ascontiguousarray(
                wbd.transpose(1, 0, 2).reshape(128, PAIRS * 128))

        # bsc = s1 * bias, split bf16 hi/lo, pair layout rows [ehi, elo, ohi, olo]
        bsc = (bias[sl].astype(np.float32) * s1c[:, None]).astype(np.float32)
        b_hi = bsc.astype(bf16)
        b_lo = (bsc - b_hi.astype(np.float32)).astype(bf16)
        b2 = np.zeros((PAIRS, 4, 128), bf16)
        b2[:, 0, :D] = b_hi[0::2]
        b2[:, 1, :D] = b_lo[0::2]
        b2[:, 2, D:] = b_hi[1::2]
        b2[:, 3, D:] = b_lo[1::2]
        b2_host = np.ascontiguousarray(
            b2.transpose(1, 0, 2).reshape(4, PAIRS * 128))

        if MM_FP16:
            in_maps.append({
                "xh": np.ascontiguousarray(xh_all[c].reshape(PAIRS, 128, B)),
                "xl": np.ascontiguousarray(xl_all[c].reshape(PAIRS, 128, B)),
                "whl": whl_host, "b2": b2_host})
        else:
            xT_c = np.ascontiguousarray(xT_all[c].reshape(PAIRS, 128, B))
            in_maps.append({"xT": xT_c, "wbd": wbd_host, "b2": b2_host})
    return in_maps


def assemble_output(results):
    """[8 cores] of yT [PAIRS, 128, B] bf16 -> full [B, N*D] fp32."""
    out = np.empty((B, N * D), np.float32)
    for c in range(NCORES):
        yy = results[c]["yT"].reshape(NS * D, B)
        ov = out[:, c * NS * D:(c + 1) * NS * D]
        for b0 in range(0, B, 128):
            ov[b0:b0 + 128, :] = yy[:, b0:b0 + 128].T.astype(np.float32)
    return out


_NC_CACHE = {}


def kernel(x, weights, bias, omega):
    x = np.ascontiguousarray(x, np.float32)
    weights = np.ascontiguousarray(weights, np.float32)
    bias = np.ascontiguousarray(bias, np.float32)
    omega = np.ascontiguousarray(omega, np.float32)

    if "nc" not in _NC_CACHE:
        _NC_CACHE["nc"] = build_bass()
    nc = _NC_CACHE["nc"]
    in_maps = prep_inputs(x, weights, bias, omega)
    res = run_bass_kernel_spmd(nc, in_maps, core_ids=list(range(NCORES)))
    return assemble_output(res.results)


# revision 19
# speedup vs baseline: 6.7667x; 1.0411x over previous
"""Trainium2 Bass kernel for nn_BlockDiagonalLayer.

Computes out[b, n*64+j] = sin(omega[n] * (sum_i x[b,n,i] * W[n,j,i] + bias[n,j]))
for B=2048, N=1024 networks, D_IN=D_OUT=64, sharded over 8 NeuronCores along N.

Per core (128 networks = 64 pairs), per pair p and batch-chunk of 1024:
  - PE:       f = Wbd_p'^T @ xT + bsc   where W' = W * (omega/2pi) is
              host-prescaled (block-diag [W'_even; W'_odd] fp32 stationary,
              all 64 preloaded in SBUF) and bsc = (omega/2pi)*bias enters via
              a tiny bf16 hi/lo bias-matmul against a ones vector.
  - DVE/ACT:  t = f + MAGIC             (rounds k = round(f) into mantissa;
              alternates DVE tensor_scalar / ACT Identity per tile)
  - DVE:      q = (t - MAGIC) - f       (= k - f, exact; scalar_tensor_tensor)
  - ACT:      y = Sin(-2pi * q) -> bf16 (|2pi q| <= pi + eps: in-table)
  sin(-2pi(k-f)) = sin(2pi f - 2pi k) = sin(omega*(Wx+b)).
Host does layout-only transforms (transpose / block-diag packing / scalars).
gpsimd (Pool/Q7) is used ONLY for setup DMAs/memsets: its elementwise
tensor ops cost ~13us per 1024-wide instruction on HW (software Q7 loop).
"""

import numpy as np
import ml_dtypes

import concourse.bass as bass
import concourse.tile as tile
from concourse import bacc, mybir
from concourse.alu_op_type import AluOpType
from concourse.bass_utils import run_bass_kernel_spmd

B, N, D = 2048, 1024, 64
NCORES = 8
NS = N // NCORES          # 128 nets per core
PAIRS = NS // 2           # 64
MMW = 512                 # matmul moving free dim (fp32 max / one PSUM bank)
EW = 1024                 # elementwise tile width (2 PSUM banks)
PB = 2                    # pairs per x/y DMA transfer

TWO_PI = float(2.0 * np.pi)
INV_2PI = float(1.0 / (2.0 * np.pi))
MAGIC = float(1.5 * 2 ** 23)

F32 = mybir.dt.float32
FP16 = mybir.dt.float16
BF16 = mybir.dt.bfloat16

MM_FP16 = True            # fp16 hi/lo 3-matmul (1 cyc/row) vs fp32 (4 cyc/row)


def build_bass(repeat: int = 1):
    """Build the per-core Bass program (same NEFF on all 8 cores).

    repeat > 1 re-runs the whole main loop (idempotent writes) for timing.
    """
    nc = bacc.Bacc("TRN2", target_bir_lowering=False, debug=False,
                   num_devices=NCORES)
    if MM_FP16:
        xh_d = nc.dram_tensor("xh", [PAIRS, 128, B], FP16, kind="ExternalInput")
        xl_d = nc.dram_tensor("xl", [PAIRS, 128, B], FP16, kind="ExternalInput")
        whl_d = nc.dram_tensor("whl", [128, PAIRS * 256], FP16,
                               kind="ExternalInput")
    else:
        xT_d = nc.dram_tensor("xT", [PAIRS, 128, B], F32, kind="ExternalInput")
        wbd_d = nc.dram_tensor("wbd", [128, PAIRS * 128], F32,
                               kind="ExternalInput")
    b2_d = nc.dram_tensor("b2", [4, PAIRS * 128], BF16, kind="ExternalInput")
    yT_d = nc.dram_tensor("yT", [PAIRS, 128, B], BF16, kind="ExternalOutput")

    with tile.TileContext(nc) as tc:
        with (
            tc.tile_pool(name="aux", bufs=1) as aux_pool,
            tc.tile_pool(name="wconst", bufs=1) as wc_pool,
            tc.tile_pool(name="xin", bufs=3) as x_pool,
            tc.tile_pool(name="oout", bufs=3) as o_pool,
            tc.tile_pool(name="tp", bufs=4) as t_pool,
            tc.tile_pool(name="qp", bufs=4) as q_pool,
            tc.tile_pool(name="ps", bufs=4, space="PSUM") as psum_pool,
        ):
            # --- constants (loaded once) ---
            if MM_FP16:
                wc_sb = wc_pool.tile([128, PAIRS * 256], FP16)
                for _c in range(8):
                    _w = PAIRS * 256 // 8
                    nc.sync.dma_start(wc_sb[:, _c * _w:(_c + 1) * _w],
                                      whl_d[:, _c * _w:(_c + 1) * _w])
            else:
                wc_sb = wc_pool.tile([128, PAIRS * 128], F32)
                for _c in range(8):
                    _w = PAIRS * 128 // 8
                    nc.sync.dma_start(wc_sb[:, _c * _w:(_c + 1) * _w],
                                      wbd_d[:, _c * _w:(_c + 1) * _w])
            b2_sb = aux_pool.tile([4, PAIRS * 128], BF16)
            nc.gpsimd.dma_start(b2_sb[:], b2_d[:])
            ones2 = aux_pool.tile([4, MMW], BF16)
            nc.gpsimd.memset(ones2[:], 1.0)
            magic_sb = aux_pool.tile([128, 1], F32)
            nc.gpsimd.memset(magic_sb[:], MAGIC)

            # --- main loop (optionally wrapped in a HW loop for timing) ---
            import contextlib
            rep_ctx = tc.For_i(0, repeat, 1) if repeat > 1 else contextlib.nullcontext()
            with rep_ctx:
                for p0 in range(0, PAIRS, PB):
                    if MM_FP16:
                        xt = x_pool.tile([128, PB * B], FP16, tag="xh")
                        nc.sync.dma_start(
                            xt[:].rearrange("p (a b) -> p a b", a=PB),
                            xh_d[p0:p0 + PB].rearrange("a p b -> p a b"))
                        xlt = x_pool.tile([128, PB * B], FP16, tag="xl")
                        nc.sync.dma_start(
                            xlt[:].rearrange("p (a b) -> p a b", a=PB),
                            xl_d[p0:p0 + PB].rearrange("a p b -> p a b"))
                    else:
                        xt = x_pool.tile([128, PB * B], F32)
                        nc.sync.dma_start(
                            xt[:].rearrange("p (a b) -> p a b", a=PB),
                            xT_d[p0:p0 + PB].rearrange("a p b -> p a b"))
                    outt = o_pool.tile([128, PB * B], BF16)
                    for a in range(PB):
                        p = p0 + a
                        b2t = b2_sb[:, p * 128:(p + 1) * 128]
                        if MM_FP16:
                            wh = wc_sb[:, p * 256:p * 256 + 128]
                            wl = wc_sb[:, p * 256 + 128:(p + 1) * 256]
                        else:
                            wst = wc_sb[:, p * 128:(p + 1) * 128]
                        for e in range(B // EW):
                            v = psum_pool.tile([128, EW], F32)
                            for h in range(EW // MMW):
                                lo = h * MMW
                                bcol = a * B + e * EW + lo
                                if MM_FP16:
                                    nc.tensor.matmul(
                                        v[:, lo:lo + MMW], wh,
                                        xt[:, bcol:bcol + MMW],
                                        start=True, stop=False)
                                    nc.tensor.matmul(
                                        v[:, lo:lo + MMW], wh,
                                        xlt[:, bcol:bcol + MMW],
                                        start=False, stop=False)
                                    nc.tensor.matmul(
                                        v[:, lo:lo + MMW], wl,
                                        xt[:, bcol:bcol + MMW],
                                        start=False, stop=False)
                                else:
                                    nc.tensor.matmul(
                                        v[:, lo:lo + MMW], wst,
                                        xt[:, bcol:bcol + MMW],
                                        start=True, stop=False)
                                nc.tensor.matmul(
                                    v[:, lo:lo + MMW], b2t, ones2[:],
                                    start=False, stop=True)
                            t = t_pool.tile([128, EW], F32)
                            # balance the magic-add: ~40% DVE, ~60% ACT
                            if (2 * p + e) % 5 < 2:
                                nc.vector.tensor_scalar_add(t[:], v[:], MAGIC)
                            else:
                                nc.scalar.activation(
                                    t[:], v[:],
                                    mybir.ActivationFunctionType.Identity,
                                    bias=magic_sb[:], scale=1.0)
                            q = q_pool.tile([128, EW], F32)
                            nc.vector.scalar_tensor_tensor(
                                q[:], t[:], MAGIC, v[:],
                                op0=AluOpType.subtract, op1=AluOpType.subtract)
                            nc.scalar.activation(
                                outt[:, a * B + e * EW:a * B + (e + 1) * EW],
                                q[:], mybir.ActivationFunctionType.Sin,
                                bias=0.0, scale=-TWO_PI)
                    nc.scalar.dma_start(
                        yT_d[p0:p0 + PB].rearrange("a p b -> p a b"),
                        outt[:].rearrange("p (a b) -> p a b", a=PB))
    nc.compile()
    return nc


def prep_inputs(x, weights, bias, omega):
    """Host-side layout prep -> list of 8 per-core input dicts."""
    bf16 = ml_dtypes.bfloat16
    x3 = x.reshape(B, NCORES, NS, D)
    # xT_all[c, n, i, b] = x[b, c*128+n, i]; blocked for cache friendliness
    xT_all = np.empty((NCORES, NS, D, B), np.float32)
    BBLK = 128
    for b0 in range(0, B, BBLK):
        xT_all[:, :, :, b0:b0 + BBLK] = x3[b0:b0 + BBLK].transpose(1, 2, 3, 0)

    s1_full = (omega.astype(np.float32) * np.float32(INV_2PI))
    if MM_FP16:
        xh_all = xT_all.astype(np.float16)
        xl_all = (xT_all - xh_all.astype(np.float32)).astype(np.float16)
    in_maps = []
    for c in range(NCORES):
        sl = slice(c * NS, (c + 1) * NS)
        s1c = s1_full[sl]                      # [128]
        # prescale: W'[n,j,i] = W[n,j,i] * s1[n]
        wc = (weights[sl] * s1c[:, None, None]).astype(np.float32)
        wT = wc.transpose(0, 2, 1)             # [net, i, j]
        if MM_FP16:
            whl = np.zeros((PAIRS, 2, 128, 128), np.float16)
            wT_h = wT.astype(np.float16)
            wT_l = (wT - wT_h.astype(np.float32)).astype(np.float16)
            whl[:, 0, :D, :D] = wT_h[0::2]
            whl[:, 0, D:, D:] = wT_h[1::2]
            whl[:, 1, :D, :D] = wT_l[0::2]
            whl[:, 1, D:, D:] = wT_l[1::2]
            whl_host = np.ascontiguousarray(
                whl.transpose(2, 0, 1, 3).reshape(128, PAIRS * 256))
        else:
            wbd = np.zeros((PAIRS, 128, 128), np.float32)
            wbd[:, :D, :D] = wT[0::2]
            wbd[:, D:, D:] = wT[1::2]
            wbd_host = np.ascontiguousarray(
                wbd.transpose(1, 0, 2).reshape(128, PAIRS * 128))

        # bsc = s1 * bias, split bf16 hi/lo, pair layout rows [ehi, elo, ohi, olo]
        bsc = (bias[sl].astype(np.float32) * s1c[:, None]).astype(np.float32)
        b_hi = bsc.astype(bf16)
        b_lo = (bsc - b_hi.astype(np.float32)).astype(bf16)
        b2 = np.zeros((PAIRS, 4, 128), bf16)
        b2[:, 0, :D] = b_hi[0::2]
        b2[:, 1, :D] = b_lo[0::2]
        b2[:, 2, D:] = b_hi[1::2]
        b2[:, 3, D:] = b_lo[1::2]
        b2_host = np.ascontiguousarray(
            b2.transpose(1, 0, 2).reshape(4, PAIRS * 128))

        if MM_FP16:
            in_maps.append({
                "xh": np.ascontiguousarray(xh_all[c].reshape(PAIRS, 128, B)),
                "xl": np.ascontiguousarray(xl_all[c].reshape(PAIRS, 128, B)),
                "whl": whl_host, "b2": b2_host})
        else:
            xT_c = np.ascontiguousarray(xT_all[c].reshape(PAIRS, 128, B))
            in_maps.append({"xT": xT_c, "wbd": wbd_host, "b2": b2_host})
    return in_maps


def assemble_output(results):
    """[8 cores] of yT [PAIRS, 128, B] bf16 -> full [B, N*D] fp32."""
    out = np.empty((B, N * D), np.float32)
    for c in range(NCORES):
        yy = results[c]["yT"].reshape(NS * D, B)
        ov = out[:, c * NS * D:(c + 1) * NS * D]
        for b0 in range(0, B, 128):
            ov[b0:b0 + 128, :] = yy[:, b0:b0 + 128].T.astype(np.float32)
    return out


_NC_CACHE = {}


def kernel(x, weights, bias, omega):
    x = np.ascontiguousarray(x, np.float32)
    weights = np.ascontiguousarray(weights, np.float32)
    bias = np.ascontiguousarray(bias, np.float32)
    omega = np.ascontiguousarray(omega, np.float32)

    if "nc" not in _NC_CACHE:
        _NC_CACHE["nc"] = build_bass()
    nc = _NC_CACHE["nc"]
    in_maps = prep_inputs(x, weights, bias, omega)
    res = run_bass_kernel_spmd(nc, in_maps, core_ids=list(range(NCORES)))
    return assemble_output(res.results)
